# revision 31
# baseline (speedup 1.0000x reference)
"""Trainium2 Bass kernel for nn_ByteSequenceEmbedder.

Packed data-parallel across 8 NeuronCores: the valid bytes of all 16
sequences (src_len ~2048 each vs T=3072 padded) are concatenated and split
at word boundaries into 8 balanced per-core streams, so each core processes
~4100 positions (9 chunks of CW<=512) instead of 2x3072 padded positions.

Stream layout per core: segments of real bytes, with
  - 2-col embed halo around mid-sequence splits (outputs discarded),
  - 2 zero cols after each sequence end (first = the reference's position
    src_len, whose conv0/highway value feeds conv1 at src_len-1),
  - a per-column multiplicative mask (cmask) applied to the conv1 input,
    zeroing the column before each sequence start (reproduces conv SAME
    zero-padding exactly).

Per-core dataflow (channels-on-partitions, buffers [128, 4*SPU] bf16):
  embed   : one-hot matmul (DVE is_equal vs per-partition iota; token ids
            host-remapped so ids >=256 land on bf16-exact even values)
  conv0   : 3 shifted matmuls per (chunk, co-chunk), ReLU evac on ACT
  highway : 2 blocks x 2 layers; 8x4 matmuls per chunk, ReLU/Sigmoid evac,
            DVE combine x' = g*(relu(h)-x)+x
  conv1   : 12 matmuls + residual add
  pool    : ragged word max-pool as masked shifted max (additive -1e30
            masks, host-built); chains split across DVE and GPSIMD
  proj    : projection over all stream cols; host gathers word-start cols

Matmul operands bf16 (f32 PSUM accumulation). Chunk layout (NCH, CW) is
computed from the actual inputs at runtime and the program is compiled for
it (cached per layout).
"""
import numpy as np

import concourse.bacc as bacc
import concourse.tile as tile
import concourse.mybir as mybir

BSZ, NW, T = 16, 1024, 3072
BED, WED = 128, 512
VOCAB = 264
BPE_MASK_IDX = 4
N_CORES = 8
BF16 = mybir.dt.bfloat16
F32 = mybir.dt.float32
F8 = mybir.dt.float8e4
_F8_NP = mybir.dt.np(mybir.dt.float8e4)
KX = 11              # x1c fp8 scale 2^KX (max|x1c|~0.016 -> ~32)

_BF16_NP = mybir.dt.np(BF16)
NEG_BIG = -1e30
NEG_F8 = -240.0      # "-inf" for fp8 masks (TRN e4m3 NaNs above 240)

_CACHE = {}


def _enc_ids(v):
    """Map token ids to bf16-exact float values (ids>=256 -> even values)."""
    v = np.asarray(v, np.int64)
    return np.where(v < 256, v, 256 + 2 * (v - 256)).astype(np.float32)


def _plan_packing(pool_lengths):
    """Balanced word-aligned split of the global byte stream across cores."""
    pl = np.asarray(pool_lengths, np.int64)
    assert pl.max() <= 3, "pool lengths > 3 unsupported"
    cum = np.cumsum(pl, axis=1)
    src = cum[:, -1]
    starts = cum - pl
    total = int(src.sum())
    seq_base = np.zeros(BSZ, np.int64)
    seq_base[1:] = np.cumsum(src)[:-1]
    gstarts = (starts + seq_base[:, None]).ravel()
    wcuts = [int(np.searchsorted(gstarts, round(total * c / N_CORES)))
             for c in range(N_CORES + 1)]
    wcuts[0], wcuts[-1] = 0, BSZ * NW

    cores = []
    max_need = 0
    for c in range(N_CORES):
        w0, w1 = wcuts[c], wcuts[c + 1]
        segs = []
        need = 0
        w = w0
        while w < w1:
            b = w // NW
            we = min(w1, (b + 1) * NW)
            lw0, lw1 = w % NW, ((we - 1) % NW) + 1
            t0, t1 = int(starts[b, lw0]), int(cum[b, lw1 - 1])
            if t1 > t0:
                segs.append((b, lw0, lw1, t0, t1))
                if t0 > 0:
                    need += min(2, t0)
                need += t1 - t0
                need += 2 if t1 == int(src[b]) else min(2, int(src[b]) - t1)
            w = we
        cores.append((segs, (w0, w1)))
        max_need = max(max_need, need)

    S8 = -(-max_need // 8) * 8
    NCH = max(1, -(-S8 // 512))
    CWmax = min(512, -(-S8 // (NCH * 8)) * 8)
    while CWmax * NCH < S8:
        NCH += 1
        CWmax = min(512, -(-S8 // (NCH * 8)) * 8)
    widths = [CWmax] * NCH
    return cores, tuple(widths), CWmax * NCH, (pl, cum, starts, src)


def _build_program(widths, kw, kwh0):
    NCH = len(widths)
    CW = max(widths)
    offs = [0]
    for w in widths:
        offs.append(offs[-1] + w)
    S = offs[-1]
    SPU = S + 16         # pitch: cols 0-7 zero, data cols 8..S+8 (16B-aligned), 8 zero cols
    nc = bacc.Bacc("TRN2", target_bir_lowering=False, debug=False)

    def dram_in(name, shape, dt):
        return nc.dram_tensor(name, shape, dt, kind="ExternalInput").ap()

    emb_lhs = dram_in("emb_lhs", [128, 3 * 128], BF16)   # tok_emb row-chunks
    emb_row4 = dram_in("emb_row4", [1, 128], BF16)       # tok_emb[4]
    iota_c = dram_in("iota_c", [128, 3], F32)            # per-partition vocab iota
    w_c0 = dram_in("w_c0", [128, 3 * WED], BF16)         # [ci, k*512+co]
    w_c1 = dram_in("w_c1", [128, 48, 128], F8)          # [(m*3+k)*2*2+..., DR pairs]
    w_hw = dram_in("w_hw", [128, 4 * 4 * 1024], BF16)    # [(bl*4+q)*1024 + co_out]
    w_hw0 = dram_in("w_hw0", [128, 32, 128], F8)         # hw0l0 DR pairs
    w_pr = dram_in("w_pr", [128, 4 * WED], BF16)         # [q*512+co]
    b_c0 = dram_in("b_c0", [128, 4], F32)
    b_c1 = dram_in("b_c1", [128, 4], F32)
    b_hw = dram_in("b_hw", [128, 4 * 8], F32)            # [bl*8 + m]
    b_pr = dram_in("b_pr", [128, 4], F32)
    tok_bc = dram_in("tok_bc", [128, S], BF16)           # encoded tokens bcast
    bpe_row = dram_in("bpe_row", [1, S], BF16)           # bpe mask 0/1
    a_msk = dram_in("a_msk", [128, 2 * S], F8)           # pooling additive masks
    c_msk = dram_in("c_msk", [128, S], BF16)             # conv1 col mask * 2^KX

    SOUT = S + ((80 - S) % 512)
    out = nc.dram_tensor("out", [WED, SOUT], BF16, kind="ExternalOutput").ap()

    RELU = mybir.ActivationFunctionType.Relu
    SIGM = mybir.ActivationFunctionType.Sigmoid
    IDEN = mybir.ActivationFunctionType.Identity
    MAX = mybir.AluOpType.max
    ADD = mybir.AluOpType.add
    SUB = mybir.AluOpType.subtract
    MUL = mybir.AluOpType.mult
    ISEQ = mybir.AluOpType.is_equal

    with tile.TileContext(nc) as tc:
        with tc.tile_pool(name="wp", bufs=1) as wp, \
             tc.tile_pool(name="ap", bufs=1) as apool, \
             tc.tile_pool(name="tp", bufs=3) as tp, \
             tc.tile_pool(name="pp", bufs=8, space="PSUM") as pp:

            # ---- HAM warm-up: PE activity from t~0 ----
            wu = wp.tile([128, 512], BF16)
            nc.vector.memset(wu[:], 0)
            for _ in range(9):
                wps = pp.tile([128, CW], F32, tag="ps", name="wps")
                nc.tensor.matmul(out=wps[:], lhsT=wu[:, 0:128], rhs=wu[:, 0:CW],
                                 start=True, stop=True)

            # ---- activation buffers (halo cols memset once; only memsets
            #      ever write them) ----
            def act_buf(tag):
                b = apool.tile([128, 4 * SPU], BF16, tag=tag, name=tag)
                for q in range(4):
                    nc.vector.memset(b[:, q * SPU:q * SPU + 8], 0)
                    nc.vector.memset(b[:, q * SPU + 8 + S:(q + 1) * SPU], 0)
                return b

            bufA = act_buf("actA")
            bufB = act_buf("actB")
            bufC = act_buf("actC")
            SPUF = S + 24        # fp8 x1c copy pitch; %16==0 for DoubleRow APs
            x1cf = apool.tile([128, 4, SPUF], F8, tag="x1cf", name="x1cf")
            for q in range(4):
                nc.vector.memset(x1cf[:, q, 0:8], 0)
                nc.vector.memset(x1cf[:, q, 8 + S:SPUF], 0)
            # aliases: tok lives in bufC (dead before hw0l1 writes bufC);
            # x0 lives in bufB slot 0 (dead before hw0l0 writes bufB)
            t_tok = bufC[:, 8:8 + S]
            x0 = bufB[:, 0:SPU]

            # first token chunk ahead of the weight loads (critical path)
            nc.sync.dma_start(out=bufC[:, 8:8 + widths[0]], in_=tok_bc[:, 0:widths[0]])

            # ---- load weights/biases once ----
            t_embA = wp.tile([128, 3 * 128], BF16)
            t_row4 = wp.tile([1, 128], BF16)
            t_iota = wp.tile([128, 3], F32)
            t_bc0 = wp.tile([128, 4], F32)
            t_bc1 = wp.tile([128, 4], F32)
            t_bhw = wp.tile([128, 4 * 8], F32)
            t_bpr = wp.tile([128, 4], F32)
            t_wc0 = wp.tile([128, 3 * WED], BF16)
            t_wc1 = wp.tile([128, 48, 128], F8)
            t_whw = wp.tile([128, 4 * 4 * 1024], BF16)
            t_whw0 = wp.tile([128, 32, 128], F8)
            t_wpr = wp.tile([128, 4 * WED], BF16)
            t_bpe = apool.tile([1, S], BF16, tag="bpe", name="t_bpe")
            t_am = apool.tile([128, 2 * S], F8, tag="am", name="t_am")
            t_cm = apool.tile([128, S], BF16, tag="cm", name="t_cm")
            for t, d in ((t_iota, iota_c), (t_embA, emb_lhs), (t_row4, emb_row4),
                         (t_bc0, b_c0), (t_bc1, b_c1), (t_bhw, b_hw), (t_bpr, b_pr),
                         (t_wc0, w_c0), (t_wc1, w_c1), (t_whw, w_hw),
                         (t_whw0, w_hw0), (t_wpr, w_pr)):
                nc.sync.dma_start(out=t[:], in_=d[:])

            scope = nc.named_scope

            # ---- embed: one-hot matmul per chunk ----
            ctx = scope("embed"); ctx.__enter__()
            for n in range(1, NCH):
                nc.scalar.dma_start(out=bufC[:, 8 + offs[n]:8 + offs[n + 1]],
                                    in_=tok_bc[:, offs[n]:offs[n + 1]])
            nc.scalar.dma_start(out=t_bpe[:], in_=bpe_row[:])
            nc.scalar.dma_start(out=t_cm[:], in_=c_msk[:])
            nc.scalar.dma_start(out=t_am[:], in_=a_msk[:])
            for n in range(NCH):
                lo, hi = offs[n], offs[n + 1]
                w = widths[n]
                oh1 = tp.tile([128, CW], BF16, tag="h", name="oh1", bufs=4)
                oh2 = tp.tile([128, CW], BF16, tag="g", name="oh2", bufs=4)
                oh3 = tp.tile([8, CW], BF16, tag="oh3", name="oh3", bufs=2)
                tb = t_tok[:, lo:hi]
                nc.vector.tensor_scalar(out=oh1[:, 0:w], in0=tb, scalar1=t_iota[:, 0:1],
                                        scalar2=None, op0=ISEQ)
                nc.vector.tensor_scalar(out=oh2[:, 0:w], in0=tb, scalar1=t_iota[:, 1:2],
                                        scalar2=None, op0=ISEQ)
                nc.vector.tensor_scalar(out=oh3[:, 0:w], in0=t_tok[0:8, lo:hi],
                                        scalar1=t_iota[0:8, 2:3], scalar2=None, op0=ISEQ)
                ps = pp.tile([128, CW], F32, tag="ps", name="ps")
                nc.tensor.matmul(out=ps[:, 0:w], lhsT=t_embA[:, 0:128], rhs=oh1[:, 0:w],
                                 start=True, stop=False)
                nc.tensor.matmul(out=ps[:, 0:w], lhsT=t_embA[:, 128:256], rhs=oh2[:, 0:w],
                                 start=False, stop=False)
                nc.tensor.matmul(out=ps[:, 0:w], lhsT=t_embA[0:8, 256:384], rhs=oh3[:, 0:w],
                                 start=False, stop=False)
                nc.tensor.matmul(out=ps[:, 0:w], lhsT=t_row4[:], rhs=t_bpe[:, lo:hi],
                                 start=False, stop=True)
                nc.scalar.activation(out=x0[:, 8 + lo:8 + hi],
                                     in_=ps[:, 0:w], func=IDEN, bias=0.0, scale=1.0)
            ctx.__exit__(None, None, None)

            # ---- conv0 ----
            ctx = scope("conv0"); ctx.__enter__()
            for n in range(NCH):
                lo, w = offs[n], widths[n]
                for m in range(4):
                    ps = pp.tile([128, CW], F32, tag="ps", name="ps")
                    for k in range(3):
                        nc.tensor.matmul(
                            out=ps[:, 0:w], lhsT=t_wc0[:, k * WED + m * 128:k * WED + (m + 1) * 128],
                            rhs=x0[:, lo + 7 + k:lo + 7 + k + w],
                            start=(k == 0), stop=(k == 2))
                    nc.scalar.activation(out=bufA[:, m * SPU + 8 + lo:m * SPU + 8 + lo + w],
                                         in_=ps[:, 0:w], func=RELU,
                                         bias=t_bc0[:, m:m + 1], scale=1.0)
                for c in range(4):
                    nc.vector.tensor_scalar(
                        out=x1cf[:, c, 8 + lo:8 + lo + w],
                        in0=bufA[:, c * SPU + 8 + lo:c * SPU + 8 + lo + w],
                        scalar1=float(2.0 ** KX), scalar2=None, op0=MUL)
            ctx.__exit__(None, None, None)

            def pool_chunk(Y, msel, n, eng_add=None):
                eng_add = eng_add or nc.vector
                """msel[t] = max(x[t], x[t+1]+A1[t], x[t+2]+A2[t]) for chunk n.
                Reads Y one/two cols into chunk n+1, so must be emitted only
                after chunk n+1's combine (engine FIFOs execute in program
                order). gpsimd only supports add/mult; max goes to DVE."""
                lo, w = offs[n], widths[n]
                hi = lo + w
                for c in range(4):
                    base = c * SPU
                    s1 = tp.tile([128, CW], BF16, tag="s1", name="s1", bufs=3)
                    s2 = tp.tile([128, CW], BF16, tag="s2", name="s2", bufs=3)
                    eng_add.tensor_tensor(out=s1[:, 0:w], in0=Y[:, base + 9 + lo:base + 9 + hi],
                                          in1=t_am[:, lo:hi], op=ADD)
                    eng_add.tensor_tensor(out=s2[:, 0:w], in0=Y[:, base + 10 + lo:base + 10 + hi],
                                          in1=t_am[:, S + lo:S + hi], op=ADD)
                    nc.vector.tensor_tensor(out=s1[:, 0:w], in0=s1[:, 0:w], in1=s2[:, 0:w], op=MAX)
                    nc.vector.tensor_tensor(out=msel[:, base + 8 + lo:base + 8 + hi],
                                            in0=s1[:, 0:w], in1=Y[:, base + 8 + lo:base + 8 + hi],
                                            op=MAX)

            def highway_layer(X, Y, bl, do_cmask=False, do_pool=False, msel=None,
                              n0=0, n1=NCH, pool_sink=None, dr=False):
                ev_scale = float(2.0 ** -(KX + kwh0)) if dr else 1.0
                """Y = g*relu(h) + (1-g)*X over chunks [n0,n1); optionally apply
                cmask to Y (conv1 input) and/or emit pool ops per chunk."""
                for n in range(n0, n1):
                    lo, w = offs[n], widths[n]
                    hi = lo + w
                    pss = []
                    for m in range(8):
                        ps = pp.tile([128, CW], F32, tag="ps", name="ps")
                        if dr:
                            for qp in range(2):
                                nc.tensor.matmul(
                                    out=ps[:, 0:w],
                                    lhsT=t_whw0[:, (m * 2 + qp) * 2:(m * 2 + qp) * 2 + 2, :],
                                    rhs=x1cf[:, 2 * qp:2 * qp + 2, 8 + lo:8 + hi],
                                    start=(qp == 0), stop=(qp == 1),
                                    perf_mode=mybir.MatmulPerfMode.DoubleRow)
                        else:
                            for q in range(4):
                                base = (bl * 4 + q) * 1024 + m * 128
                                nc.tensor.matmul(
                                    out=ps[:, 0:w], lhsT=t_whw[:, base:base + 128],
                                    rhs=X[:, q * SPU + 8 + lo:q * SPU + 8 + hi],
                                    start=(q == 0), stop=(q == 3))
                        pss.append(ps)
                    for c in range(4):
                        xs = X[:, c * SPU + 8 + lo:c * SPU + 8 + hi]
                        h_t = tp.tile([128, CW], BF16, tag="h", name="h_t", bufs=4)
                        g_t = tp.tile([128, CW], BF16, tag="g", name="g_t", bufs=4)
                        d_t = tp.tile([128, CW], BF16, tag="d", name="d_t", bufs=2)
                        nc.scalar.activation(out=h_t[:, 0:w], in_=pss[c][:, 0:w], func=RELU,
                                             bias=t_bhw[:, bl * 8 + c:bl * 8 + c + 1], scale=ev_scale)
                        nc.scalar.activation(out=g_t[:, 0:w], in_=pss[4 + c][:, 0:w], func=SIGM,
                                             bias=t_bhw[:, bl * 8 + 4 + c:bl * 8 + 4 + c + 1], scale=ev_scale)
                        nc.vector.tensor_tensor(out=d_t[:, 0:w], in0=h_t[:, 0:w], in1=xs, op=SUB)
                        nc.vector.tensor_tensor(out=d_t[:, 0:w], in0=d_t[:, 0:w], in1=g_t[:, 0:w], op=MUL)
                        ys = Y[:, c * SPU + 8 + lo:c * SPU + 8 + hi]
                        nc.vector.tensor_tensor(out=ys, in0=d_t[:, 0:w], in1=xs, op=ADD)
                        if do_cmask:
                            # fused cmask + 2^KX scale + fp8 cast for conv1's
                            # rhs; bf16 x1c stays unmasked (residual only uses
                            # body cols)
                            nc.vector.tensor_tensor(
                                out=x1cf[:, c, 8 + lo:8 + hi], in0=ys,
                                in1=t_cm[:, lo:hi], op=MUL)
                    if do_pool and n1 == NCH and n >= 1:
                        pool_chunk(Y, msel, n - 1, eng_add=nc.gpsimd)
                if do_pool and n1 == NCH:
                    pool_chunk(Y, msel, NCH - 1, eng_add=nc.gpsimd)
                if do_pool and n1 != NCH:
                    for n in range(max(n0, 1), n1):
                        pool_sink.append(n - 1)
                if do_pool and n1 == NCH:
                    # last pass: inline per-chunk pools with gpsimd ADDs (DVE
                    # carries only combines + maxes, and the drain starts a
                    # pass earlier for proj)
                    pass

            def x1cf_copy(n):
                lo, w = offs[n], widths[n]
                for c in range(4):
                    nc.vector.tensor_scalar(
                        out=x1cf[:, c, 8 + lo:8 + lo + w],
                        in0=bufC[:, c * SPU + 8 + lo:c * SPU + 8 + lo + w],
                        scalar1=float(2.0 ** KX), scalar2=None, op0=MUL)

            with scope("hw0l0"):
                highway_layer(bufA, bufB, 0, dr=True)
            with scope("hw0l1"):
                highway_layer(bufB, bufC, 1, do_cmask=True)

            # ---- conv1 (+res) ----
            def conv1_range(n0, n1):
              for n in range(n0, n1):
                lo, w = offs[n], widths[n]
                for m in range(4):
                    ps = pp.tile([128, CW], F32, tag="ps", name="ps")
                    i = 0
                    for k in range(3):
                        for qp in range(2):
                            j = ((m * 3 + k) * 2 + qp) * 2
                            nc.tensor.matmul(
                                out=ps[:, 0:w], lhsT=t_wc1[:, j:j + 2, :],
                                rhs=x1cf[:, 2 * qp:2 * qp + 2, 7 + lo + k:7 + lo + k + w],
                                start=(i == 0), stop=(i == 5),
                                perf_mode=mybir.MatmulPerfMode.DoubleRow)
                            i += 1
                    r_t = tp.tile([128, CW], BF16, tag="h", name="r_t", bufs=4)
                    nc.scalar.activation(out=r_t[:, 0:w], in_=ps[:, 0:w], func=RELU,
                                         bias=t_bc1[:, m:m + 1],
                                         scale=float(2.0 ** -(KX + kw)))
                    xs = bufC[:, m * SPU + 8 + lo:m * SPU + 8 + lo + w]
                    nc.vector.tensor_tensor(
                        out=bufA[:, m * SPU + 8 + lo:m * SPU + 8 + lo + w],
                        in0=r_t[:, 0:w], in1=xs, op=ADD)

            # Multi-pass pipeline: pool work of each pass drains into the
            # next pass's conv1 PE window (DVE/GPSIMD are idle there). Each
            # hw1l1 pass stops one chunk short of the pass boundary: the next
            # conv1 pass still needs x1c chunk b-1's last col (hw1l1 would
            # overwrite it with x2).
            PB = [0, max(1, NCH - 6), max(2, NCH - 4), max(3, NCH - 2), NCH]
            NP = len(PB) - 1
            deferred = []
            for i in range(NP):
                b0, b1 = PB[i], PB[i + 1]
                with scope(f"conv1_{i}"):
                    conv1_range(b0, b1)
                    for pn in deferred:
                        pool_chunk(bufC, bufA, pn, eng_add=nc.gpsimd)
                    deferred = []
                with scope(f"hw1l0_{i}"):
                    highway_layer(bufA, bufB, 2, n0=b0, n1=b1)
                h0 = b0 - 1 if i > 0 else 0
                h1 = b1 - 1 if i < NP - 1 else NCH
                with scope(f"hw1l1_{i}"):
                    highway_layer(bufB, bufC, 3, do_pool=True, msel=bufA,
                                  n0=h0, n1=h1, pool_sink=deferred)
            with scope("pool_tail"):
                for pn in deferred:
                    pool_chunk(bufC, bufA, pn, eng_add=nc.gpsimd)

            # ---- projection over all stream cols ----
            ctx = scope("proj"); ctx.__enter__()
            for n in range(NCH):
                lo, w = offs[n], widths[n]
                hi = lo + w
                for m in range(4):
                    ps = pp.tile([128, CW], F32, tag="ps", name="ps")
                    for q in range(4):
                        nc.tensor.matmul(
                            out=ps[:, 0:w], lhsT=t_wpr[:, q * WED + m * 128:q * WED + (m + 1) * 128],
                            rhs=bufA[:, q * SPU + 8 + lo:q * SPU + 8 + hi],
                            start=(q == 0), stop=(q == 3))
                    o_t = tp.tile([128, CW], BF16, tag="o", name="o_t", bufs=3)
                    if n == NCH - 1 and m % 2 == 1:
                        # last chunk: split evacs across ACT and (idle) DVE to
                        # shorten the post-matmul tail
                        nc.vector.tensor_scalar(out=o_t[:, 0:w], in0=ps[:, 0:w],
                                                scalar1=t_bpr[:, m:m + 1],
                                                scalar2=None, op0=ADD)
                    else:
                        nc.scalar.activation(out=o_t[:, 0:w], in_=ps[:, 0:w], func=IDEN,
                                             bias=t_bpr[:, m:m + 1], scale=1.0)
                    dq = nc.sync if (n * 4 + m) % 2 == 0 else nc.scalar
                    dq.dma_start(out=out[m * 128:(m + 1) * 128, lo:hi], in_=o_t[:, 0:w])
            ctx.__exit__(None, None, None)

    nc.compile()
    return nc


def _prep_inputs(inputs):
    """Host-side: pack + shard + convert to the kernel's DRAM layouts."""
    byte_tokens = np.asarray(inputs["byte_tokens"], np.int64)
    bpe_mask = np.asarray(inputs["bpe_mask"], bool)
    pool_lengths = np.asarray(inputs["pool_lengths"], np.int64)
    tok_emb = np.asarray(inputs["tok_emb"], np.float32)

    cores, widths, S, (pl, cum, starts, src) = _plan_packing(pool_lengths)

    def bf(x):
        return np.ascontiguousarray(np.asarray(x, np.float32).astype(_BF16_NP))

    conv0_W = np.asarray(inputs["conv0_W"], np.float32)   # [3,128,512]
    conv1_W = np.asarray(inputs["conv1_W"], np.float32)   # [3,512,512]
    hw0_W = np.asarray(inputs["hw0_W"], np.float32)       # [2,1024,512]
    hw1_W = np.asarray(inputs["hw1_W"], np.float32)
    proj_W = np.asarray(inputs["proj_W"], np.float32)     # [512,512]

    w_c0 = bf(conv0_W.transpose(1, 0, 2).reshape(128, 3 * WED))
    kw = int(np.floor(np.log2(128.0 / max(np.abs(conv1_W).max(), 1e-30))))
    w_c1 = np.empty((128, 48, 128), np.float32)
    for m in range(4):
        for k in range(3):
            for q in range(4):
                j = ((m * 3 + k) * 2 + (q // 2)) * 2 + (q % 2)
                w_c1[:, j, :] = conv1_W[k, q * 128:(q + 1) * 128, m * 128:(m + 1) * 128]
    w_c1 = np.ascontiguousarray((w_c1 * 2.0 ** kw).astype(_F8_NP))
    whw = np.empty((128, 16, 1024), np.float32)
    for bl, (blk, lay) in enumerate(((hw0_W, 0), (hw0_W, 1), (hw1_W, 0), (hw1_W, 1))):
        wt = blk[lay].T  # [512, 1024]
        for q in range(4):
            whw[:, bl * 4 + q, :] = wt[q * 128:(q + 1) * 128]
    w_hw = bf(whw.reshape(128, 16 * 1024))
    kwh0 = int(np.floor(np.log2(128.0 / max(np.abs(hw0_W[0]).max(), 1e-30))))
    w_hw0 = np.empty((128, 32, 128), np.float32)
    for m in range(8):
        for q in range(4):
            w_hw0[:, m * 4 + q, :] = hw0_W[0, m * 128:(m + 1) * 128, q * 128:(q + 1) * 128].T
    w_hw0 = np.ascontiguousarray((w_hw0 * 2.0 ** kwh0).astype(_F8_NP))
    w_pr = bf(proj_W.T.reshape(4, 128, WED).transpose(1, 0, 2).reshape(128, 4 * WED))

    def colchunks(b):  # [512] -> [128, 4]
        return np.ascontiguousarray(np.asarray(b, np.float32).reshape(4, 128).T)

    b_c0 = colchunks(inputs["conv0_b"])
    b_c1 = colchunks(inputs["conv1_b"])
    bhw = np.empty((128, 4, 8), np.float32)
    for bl, (blk, lay) in enumerate((("hw0_b", 0), ("hw0_b", 1), ("hw1_b", 0), ("hw1_b", 1))):
        b = np.asarray(inputs[blk], np.float32)[lay]      # [1024]
        bhw[:, bl, 0:4] = b[:512].reshape(4, 128).T
        bhw[:, bl, 4:8] = b[512:1024].reshape(4, 128).T
    b_hw = np.ascontiguousarray(bhw.reshape(128, 32))
    b_pr = colchunks(inputs["proj_b"])

    emb_lhs = np.zeros((128, 3 * 128), np.float32)
    emb_lhs[:, 0:128] = tok_emb[0:128]
    emb_lhs[:, 128:256] = tok_emb[128:256]
    emb_lhs[0:8, 256:384] = tok_emb[256:264]
    emb_lhs = bf(emb_lhs)
    emb_row4 = bf(tok_emb[BPE_MASK_IDX:BPE_MASK_IDX + 1, :])  # [1, 128]
    iota_c = np.empty((128, 3), np.float32)
    p = np.arange(128)
    iota_c[:, 0] = p
    iota_c[:, 1] = 128 + p
    iota_c[:, 2] = _enc_ids(256 + p)   # only partitions 0..7 used

    shared = dict(emb_lhs=emb_lhs, emb_row4=emb_row4, iota_c=iota_c,
                  w_c0=w_c0, w_c1=w_c1, w_hw=w_hw, w_hw0=w_hw0, w_pr=w_pr,
                  b_c0=b_c0, b_c1=b_c1, b_hw=b_hw, b_pr=b_pr)

    in_maps = []
    meta = []
    for core in range(N_CORES):
        segs, _wr = cores[core]
        tok = np.zeros(S, np.float32)
        bpe = np.zeros(S, np.float32)
        a1 = np.full(S, NEG_BIG, np.float32)
        a2 = np.full(S, NEG_BIG, np.float32)
        cmk = np.zeros(S, np.float32)
        wrows, wcols = [], []
        pos = 0
        for (b, lw0, lw1, t0, t1) in segs:
            if t0 > 0:
                hl = min(2, t0)
                tok[pos:pos + hl] = _enc_ids(byte_tokens[b, t0 - hl:t0])
                bpe[pos:pos + hl] = bpe_mask[b, t0 - hl:t0]
                cmk[pos:pos + hl] = 1.0
                pos += hl
            body = pos
            nb = t1 - t0
            tok[pos:pos + nb] = _enc_ids(byte_tokens[b, t0:t1])
            bpe[pos:pos + nb] = bpe_mask[b, t0:t1]
            cmk[pos:pos + nb] = 1.0
            lw = np.arange(lw0, lw1)
            ln = pl[b, lw0:lw1]
            wst = starts[b, lw0:lw1] - t0 + body
            nz = ln > 0
            a1[wst[nz]] = np.where(ln[nz] > 1, 0.0, NEG_BIG)
            a2[wst[nz]] = np.where(ln[nz] > 2, 0.0, NEG_BIG)
            wrows.append(b * NW + lw[nz])
            wcols.append(wst[nz])
            pos += nb
            if t1 == int(src[b]):
                cmk[pos] = 1.0      # gap1: reference position src_len
                pos += 2
            else:
                hr = min(2, int(src[b]) - t1)
                tok[pos:pos + hr] = _enc_ids(byte_tokens[b, t1:t1 + hr])
                bpe[pos:pos + hr] = bpe_mask[b, t1:t1 + hr]
                cmk[pos] = 1.0
                pos += hr
        assert pos <= S, (pos, S)

        m = dict(shared)
        m["tok_bc"] = np.ascontiguousarray(
            np.broadcast_to(tok.astype(_BF16_NP), (128, S)))
        m["bpe_row"] = bpe.astype(_BF16_NP).reshape(1, S)
        am = np.concatenate([a1, a2])
        am = np.where(am < 0, NEG_F8, 0.0).astype(_F8_NP)
        m["a_msk"] = np.ascontiguousarray(np.broadcast_to(am, (128, 2 * S)))
        m["c_msk"] = np.ascontiguousarray(
            np.broadcast_to((cmk * 2.0 ** KX).astype(_BF16_NP), (128, S)))
        in_maps.append(m)
        meta.append((np.concatenate(wrows) if wrows else np.empty(0, np.int64),
                     np.concatenate(wcols) if wcols else np.empty(0, np.int64)))
    return in_maps, (meta, widths, kw, kwh0)


def kernel(**inputs) -> np.ndarray:
    from concourse.bass_utils import run_bass_kernel_spmd

    in_maps, (meta, widths, kw, kwh0) = _prep_inputs(inputs)
    key = (widths, kw, kwh0)
    if _CACHE.get("key") != key:
        _CACHE["nc"] = _build_program(widths, kw, kwh0)
        _CACHE["key"] = key
    nc = _CACHE["nc"]

    res = run_bass_kernel_spmd(nc, in_maps, list(range(N_CORES)))

    proj_b = np.asarray(inputs["proj_b"], np.float32)
    full = np.empty((BSZ * NW, WED), np.float32)
    full[:] = proj_b
    for core in range(N_CORES):
        o = np.asarray(res.results[core]["out"], np.float32)  # [512, S]
        rows, cols = meta[core]
        if len(rows):
            full[rows] = o[:, cols].T
    return full.reshape(BSZ, NW, WED)


# revision 32
# speedup vs baseline: 1.0470x; 1.0470x over previous
"""Trainium2 Bass kernel for nn_ByteSequenceEmbedder.

Packed data-parallel across 8 NeuronCores: the valid bytes of all 16
sequences (src_len ~2048 each vs T=3072 padded) are concatenated and split
at word boundaries into 8 balanced per-core streams, so each core processes
~4100 positions (9 chunks of CW<=512) instead of 2x3072 padded positions.

Stream layout per core: segments of real bytes, with
  - 2-col embed halo around mid-sequence splits (outputs discarded),
  - 2 zero cols after each sequence end (first = the reference's position
    src_len, whose conv0/highway value feeds conv1 at src_len-1),
  - a per-column multiplicative mask (cmask) applied to the conv1 input,
    zeroing the column before each sequence start (reproduces conv SAME
    zero-padding exactly).

Per-core dataflow (channels-on-partitions, buffers [128, 4*SPU] bf16):
  embed   : one-hot matmul (DVE is_equal vs per-partition iota; token ids
            host-remapped so ids >=256 land on bf16-exact even values)
  conv0   : 3 shifted matmuls per (chunk, co-chunk), ReLU evac on ACT
  highway : 2 blocks x 2 layers; 8x4 matmuls per chunk, ReLU/Sigmoid evac,
            DVE combine x' = g*(relu(h)-x)+x
  conv1   : 12 matmuls + residual add
  pool    : ragged word max-pool as masked shifted max (additive -1e30
            masks, host-built); chains split across DVE and GPSIMD
  proj    : projection over all stream cols; host gathers word-start cols

Matmul operands bf16 (f32 PSUM accumulation). Chunk layout (NCH, CW) is
computed from the actual inputs at runtime and the program is compiled for
it (cached per layout).
"""
import numpy as np

import concourse.bacc as bacc
import concourse.tile as tile
import concourse.mybir as mybir

BSZ, NW, T = 16, 1024, 3072
BED, WED = 128, 512
VOCAB = 264
BPE_MASK_IDX = 4
N_CORES = 8
BF16 = mybir.dt.bfloat16
F32 = mybir.dt.float32
F8 = mybir.dt.float8e4
_F8_NP = mybir.dt.np(mybir.dt.float8e4)
KX = 11              # x1c fp8 scale 2^KX (max|x1c|~0.016 -> ~32)

_BF16_NP = mybir.dt.np(BF16)
NEG_BIG = -1e30
NEG_F8 = -240.0      # "-inf" for fp8 masks (TRN e4m3 NaNs above 240)

_CACHE = {}


def _enc_ids(v):
    """Map token ids to bf16-exact float values (ids>=256 -> even values)."""
    v = np.asarray(v, np.int64)
    return np.where(v < 256, v, 256 + 2 * (v - 256)).astype(np.float32)


def _plan_packing(pool_lengths):
    """Balanced word-aligned split of the global byte stream across cores."""
    pl = np.asarray(pool_lengths, np.int64)
    assert pl.max() <= 3, "pool lengths > 3 unsupported"
    cum = np.cumsum(pl, axis=1)
    src = cum[:, -1]
    starts = cum - pl
    total = int(src.sum())
    seq_base = np.zeros(BSZ, np.int64)
    seq_base[1:] = np.cumsum(src)[:-1]
    gstarts = (starts + seq_base[:, None]).ravel()
    wcuts = [int(np.searchsorted(gstarts, round(total * c / N_CORES)))
             for c in range(N_CORES + 1)]
    wcuts[0], wcuts[-1] = 0, BSZ * NW

    cores = []
    max_need = 0
    for c in range(N_CORES):
        w0, w1 = wcuts[c], wcuts[c + 1]
        segs = []
        need = 0
        w = w0
        while w < w1:
            b = w // NW
            we = min(w1, (b + 1) * NW)
            lw0, lw1 = w % NW, ((we - 1) % NW) + 1
            t0, t1 = int(starts[b, lw0]), int(cum[b, lw1 - 1])
            if t1 > t0:
                segs.append((b, lw0, lw1, t0, t1))
                if t0 > 0:
                    need += min(2, t0)
                need += t1 - t0
                need += 2 if t1 == int(src[b]) else min(2, int(src[b]) - t1)
            w = we
        cores.append((segs, (w0, w1)))
        max_need = max(max_need, need)

    S8 = -(-max_need // 8) * 8
    NCH = max(1, -(-S8 // 512))
    CWmax = min(512, -(-S8 // (NCH * 8)) * 8)
    while CWmax * NCH < S8:
        NCH += 1
        CWmax = min(512, -(-S8 // (NCH * 8)) * 8)
    widths = [CWmax] * NCH
    return cores, tuple(widths), CWmax * NCH, (pl, cum, starts, src)


def _build_program(widths, kw, kwh0):
    NCH = len(widths)
    CW = max(widths)
    offs = [0]
    for w in widths:
        offs.append(offs[-1] + w)
    S = offs[-1]
    SPU = S + 16         # pitch: cols 0-7 zero, data cols 8..S+8 (16B-aligned), 8 zero cols
    nc = bacc.Bacc("TRN2", target_bir_lowering=False, debug=False)

    def dram_in(name, shape, dt):
        return nc.dram_tensor(name, shape, dt, kind="ExternalInput").ap()

    emb_lhs = dram_in("emb_lhs", [128, 3 * 128], BF16)   # tok_emb row-chunks
    emb_row4 = dram_in("emb_row4", [1, 128], BF16)       # tok_emb[4]
    iota_c = dram_in("iota_c", [128, 3], F32)            # per-partition vocab iota
    w_c0 = dram_in("w_c0", [128, 3 * WED], BF16)         # [ci, k*512+co]
    w_c1 = dram_in("w_c1", [128, 48, 128], F8)          # [(m*3+k)*2*2+..., DR pairs]
    w_hw = dram_in("w_hw", [128, 4 * 4 * 1024], BF16)    # [(bl*4+q)*1024 + co_out]
    w_hw0 = dram_in("w_hw0", [128, 32, 128], F8)         # hw0l0 DR pairs
    w_pr = dram_in("w_pr", [128, 4 * WED], BF16)         # [q*512+co]
    b_c0 = dram_in("b_c0", [128, 4], F32)
    b_c1 = dram_in("b_c1", [128, 4], F32)
    b_hw = dram_in("b_hw", [128, 4 * 8], F32)            # [bl*8 + m]
    b_pr = dram_in("b_pr", [128, 4], F32)
    tok_bc = dram_in("tok_bc", [128, S], BF16)           # encoded tokens bcast
    bpe_row = dram_in("bpe_row", [1, S], BF16)           # bpe mask 0/1
    a_msk = dram_in("a_msk", [128, 2 * S], F8)           # pooling additive masks
    c_msk = dram_in("c_msk", [128, S], BF16)             # conv1 col mask * 2^KX

    SOUT = S + ((80 - S) % 512)
    out = nc.dram_tensor("out", [WED, SOUT], BF16, kind="ExternalOutput").ap()

    RELU = mybir.ActivationFunctionType.Relu
    SIGM = mybir.ActivationFunctionType.Sigmoid
    IDEN = mybir.ActivationFunctionType.Identity
    MAX = mybir.AluOpType.max
    ADD = mybir.AluOpType.add
    SUB = mybir.AluOpType.subtract
    MUL = mybir.AluOpType.mult
    ISEQ = mybir.AluOpType.is_equal

    with tile.TileContext(nc) as tc:
        with tc.tile_pool(name="wp", bufs=1) as wp, \
             tc.tile_pool(name="ap", bufs=1) as apool, \
             tc.tile_pool(name="tp", bufs=3) as tp, \
             tc.tile_pool(name="pp", bufs=8, space="PSUM") as pp:

            # ---- HAM warm-up: PE activity from t~0 ----
            wu = wp.tile([128, 512], BF16)
            nc.vector.memset(wu[:], 0)
            for _ in range(9):
                wps = pp.tile([128, CW], F32, tag="ps", name="wps")
                nc.tensor.matmul(out=wps[:], lhsT=wu[:, 0:128], rhs=wu[:, 0:CW],
                                 start=True, stop=True)

            # ---- activation buffers (halo cols memset once; only memsets
            #      ever write them) ----
            def act_buf(tag):
                b = apool.tile([128, 4 * SPU], BF16, tag=tag, name=tag)
                for q in range(4):
                    nc.vector.memset(b[:, q * SPU:q * SPU + 8], 0)
                    nc.vector.memset(b[:, q * SPU + 8 + S:(q + 1) * SPU], 0)
                return b

            bufA = act_buf("actA")
            bufB = act_buf("actB")
            bufC = act_buf("actC")
            SPUF = S + 24        # fp8 x1c copy pitch; %16==0 for DoubleRow APs
            x1cf = apool.tile([128, 4, SPUF], F8, tag="x1cf", name="x1cf")
            for q in range(4):
                nc.vector.memset(x1cf[:, q, 0:8], 0)
                nc.vector.memset(x1cf[:, q, 8 + S:SPUF], 0)
            # aliases: tok lives in bufC (dead before hw0l1 writes bufC);
            # x0 lives in bufB slot 0 (dead before hw0l0 writes bufB)
            t_tok = bufC[:, 8:8 + S]
            x0 = bufB[:, 0:SPU]

            # first token chunk ahead of the weight loads (critical path)
            nc.sync.dma_start(out=bufC[:, 8:8 + widths[0]], in_=tok_bc[:, 0:widths[0]])

            # ---- load weights/biases once ----
            t_embA = wp.tile([128, 3 * 128], BF16)
            t_row4 = wp.tile([1, 128], BF16)
            t_iota = wp.tile([128, 3], F32)
            t_bc0 = wp.tile([128, 4], F32)
            t_bc1 = wp.tile([128, 4], F32)
            t_bhw = wp.tile([128, 4 * 8], F32)
            t_bpr = wp.tile([128, 4], F32)
            t_wc0 = wp.tile([128, 3 * WED], BF16)
            t_wc1 = wp.tile([128, 48, 128], F8)
            t_whw = wp.tile([128, 4 * 4 * 1024], BF16)
            t_whw0 = wp.tile([128, 32, 128], F8)
            t_wpr = wp.tile([128, 4 * WED], BF16)
            t_bpe = apool.tile([1, S], BF16, tag="bpe", name="t_bpe")
            t_am = apool.tile([128, 2 * S], F8, tag="am", name="t_am")
            t_cm = apool.tile([128, S], BF16, tag="cm", name="t_cm")
            for t, d in ((t_iota, iota_c), (t_embA, emb_lhs), (t_row4, emb_row4),
                         (t_bc0, b_c0), (t_bc1, b_c1), (t_bhw, b_hw), (t_bpr, b_pr),
                         (t_wc0, w_c0), (t_wc1, w_c1), (t_whw, w_hw),
                         (t_whw0, w_hw0), (t_wpr, w_pr)):
                nc.sync.dma_start(out=t[:], in_=d[:])

            scope = nc.named_scope

            # ---- embed: one-hot matmul per chunk ----
            ctx = scope("embed"); ctx.__enter__()
            for n in range(1, NCH):
                nc.scalar.dma_start(out=bufC[:, 8 + offs[n]:8 + offs[n + 1]],
                                    in_=tok_bc[:, offs[n]:offs[n + 1]])
            nc.scalar.dma_start(out=t_bpe[:], in_=bpe_row[:])
            nc.scalar.dma_start(out=t_cm[:], in_=c_msk[:])
            nc.scalar.dma_start(out=t_am[:], in_=a_msk[:])
            for n in range(NCH):
                lo, hi = offs[n], offs[n + 1]
                w = widths[n]
                oh1 = tp.tile([128, CW], BF16, tag="h", name="oh1", bufs=4)
                oh2 = tp.tile([128, CW], BF16, tag="g", name="oh2", bufs=4)
                oh3 = tp.tile([8, CW], BF16, tag="oh3", name="oh3", bufs=2)
                tb = t_tok[:, lo:hi]
                nc.vector.tensor_scalar(out=oh1[:, 0:w], in0=tb, scalar1=t_iota[:, 0:1],
                                        scalar2=None, op0=ISEQ)
                nc.vector.tensor_scalar(out=oh2[:, 0:w], in0=tb, scalar1=t_iota[:, 1:2],
                                        scalar2=None, op0=ISEQ)
                nc.vector.tensor_scalar(out=oh3[:, 0:w], in0=t_tok[0:8, lo:hi],
                                        scalar1=t_iota[0:8, 2:3], scalar2=None, op0=ISEQ)
                ps = pp.tile([128, CW], F32, tag="ps", name="ps")
                nc.tensor.matmul(out=ps[:, 0:w], lhsT=t_embA[:, 0:128], rhs=oh1[:, 0:w],
                                 start=True, stop=False)
                nc.tensor.matmul(out=ps[:, 0:w], lhsT=t_embA[:, 128:256], rhs=oh2[:, 0:w],
                                 start=False, stop=False)
                nc.tensor.matmul(out=ps[:, 0:w], lhsT=t_embA[0:8, 256:384], rhs=oh3[:, 0:w],
                                 start=False, stop=False)
                nc.tensor.matmul(out=ps[:, 0:w], lhsT=t_row4[:], rhs=t_bpe[:, lo:hi],
                                 start=False, stop=True)
                nc.scalar.activation(out=x0[:, 8 + lo:8 + hi],
                                     in_=ps[:, 0:w], func=IDEN, bias=0.0, scale=1.0)
            ctx.__exit__(None, None, None)

            # ---- conv0 ----
            ctx = scope("conv0"); ctx.__enter__()
            for n in range(NCH):
                lo, w = offs[n], widths[n]
                for m in range(4):
                    ps = pp.tile([128, CW], F32, tag="ps", name="ps")
                    for k in range(3):
                        nc.tensor.matmul(
                            out=ps[:, 0:w], lhsT=t_wc0[:, k * WED + m * 128:k * WED + (m + 1) * 128],
                            rhs=x0[:, lo + 7 + k:lo + 7 + k + w],
                            start=(k == 0), stop=(k == 2))
                    nc.scalar.activation(out=bufA[:, m * SPU + 8 + lo:m * SPU + 8 + lo + w],
                                         in_=ps[:, 0:w], func=RELU,
                                         bias=t_bc0[:, m:m + 1], scale=1.0)
                for c in range(4):
                    nc.vector.tensor_scalar(
                        out=x1cf[:, c, 8 + lo:8 + lo + w],
                        in0=bufA[:, c * SPU + 8 + lo:c * SPU + 8 + lo + w],
                        scalar1=float(2.0 ** KX), scalar2=None, op0=MUL)
            ctx.__exit__(None, None, None)

            def pool_chunk(Y, msel, n, eng_add=None):
                eng_add = eng_add or nc.vector
                """msel[t] = max(x[t], x[t+1]+A1[t], x[t+2]+A2[t]) for chunk n.
                Reads Y one/two cols into chunk n+1, so must be emitted only
                after chunk n+1's combine (engine FIFOs execute in program
                order). gpsimd only supports add/mult; max goes to DVE."""
                lo, w = offs[n], widths[n]
                hi = lo + w
                for c in range(4):
                    base = c * SPU
                    s1 = tp.tile([128, CW], BF16, tag="s1", name="s1", bufs=3)
                    s2 = tp.tile([128, CW], BF16, tag="s2", name="s2", bufs=3)
                    eng_add.tensor_tensor(out=s1[:, 0:w], in0=Y[:, base + 9 + lo:base + 9 + hi],
                                          in1=t_am[:, lo:hi], op=ADD)
                    eng_add.tensor_tensor(out=s2[:, 0:w], in0=Y[:, base + 10 + lo:base + 10 + hi],
                                          in1=t_am[:, S + lo:S + hi], op=ADD)
                    nc.vector.tensor_tensor(out=s1[:, 0:w], in0=s1[:, 0:w], in1=s2[:, 0:w], op=MAX)
                    nc.vector.tensor_tensor(out=msel[:, base + 8 + lo:base + 8 + hi],
                                            in0=s1[:, 0:w], in1=Y[:, base + 8 + lo:base + 8 + hi],
                                            op=MAX)

            def highway_layer(X, Y, bl, do_cmask=False, do_pool=False, msel=None,
                              n0=0, n1=NCH, pool_sink=None, dr=False):
                ev_scale = float(2.0 ** -(KX + kwh0)) if dr else 1.0
                """Y = g*relu(h) + (1-g)*X over chunks [n0,n1); optionally apply
                cmask to Y (conv1 input) and/or emit pool ops per chunk."""
                for n in range(n0, n1):
                    lo, w = offs[n], widths[n]
                    hi = lo + w
                    pss = []
                    for m in range(8):
                        ps = pp.tile([128, CW], F32, tag="ps", name="ps")
                        if dr:
                            for qp in range(2):
                                nc.tensor.matmul(
                                    out=ps[:, 0:w],
                                    lhsT=t_whw0[:, (m * 2 + qp) * 2:(m * 2 + qp) * 2 + 2, :],
                                    rhs=x1cf[:, 2 * qp:2 * qp + 2, 8 + lo:8 + hi],
                                    start=(qp == 0), stop=(qp == 1),
                                    perf_mode=mybir.MatmulPerfMode.DoubleRow)
                        else:
                            for q in range(4):
                                base = (bl * 4 + q) * 1024 + m * 128
                                nc.tensor.matmul(
                                    out=ps[:, 0:w], lhsT=t_whw[:, base:base + 128],
                                    rhs=X[:, q * SPU + 8 + lo:q * SPU + 8 + hi],
                                    start=(q == 0), stop=(q == 3))
                        pss.append(ps)
                    for c in range(4):
                        xs = X[:, c * SPU + 8 + lo:c * SPU + 8 + hi]
                        h_t = tp.tile([128, CW], BF16, tag="h", name="h_t", bufs=4)
                        g_t = tp.tile([128, CW], BF16, tag="g", name="g_t", bufs=4)
                        d_t = tp.tile([128, CW], BF16, tag="d", name="d_t", bufs=2)
                        nc.scalar.activation(out=h_t[:, 0:w], in_=pss[c][:, 0:w], func=RELU,
                                             bias=t_bhw[:, bl * 8 + c:bl * 8 + c + 1], scale=ev_scale)
                        nc.scalar.activation(out=g_t[:, 0:w], in_=pss[4 + c][:, 0:w], func=SIGM,
                                             bias=t_bhw[:, bl * 8 + 4 + c:bl * 8 + 4 + c + 1], scale=ev_scale)
                        nc.vector.tensor_tensor(out=d_t[:, 0:w], in0=h_t[:, 0:w], in1=xs, op=SUB)
                        nc.vector.tensor_tensor(out=d_t[:, 0:w], in0=d_t[:, 0:w], in1=g_t[:, 0:w], op=MUL)
                        ys = Y[:, c * SPU + 8 + lo:c * SPU + 8 + hi]
                        nc.vector.tensor_tensor(out=ys, in0=d_t[:, 0:w], in1=xs, op=ADD)
                        if do_cmask:
                            # fused cmask + 2^KX scale + fp8 cast for conv1's
                            # rhs; bf16 x1c stays unmasked (residual only uses
                            # body cols)
                            nc.vector.tensor_tensor(
                                out=x1cf[:, c, 8 + lo:8 + hi], in0=ys,
                                in1=t_cm[:, lo:hi], op=MUL)
                    if do_pool and n1 == NCH and n >= 1:
                        pool_chunk(Y, msel, n - 1, eng_add=nc.gpsimd)
                if do_pool and n1 == NCH:
                    pool_chunk(Y, msel, NCH - 1, eng_add=nc.gpsimd)
                if do_pool and n1 != NCH:
                    for n in range(max(n0, 1), n1):
                        pool_sink.append(n - 1)
                if do_pool and n1 == NCH:
                    # last pass: inline per-chunk pools with gpsimd ADDs (DVE
                    # carries only combines + maxes, and the drain starts a
                    # pass earlier for proj)
                    pass

            def x1cf_copy(n):
                lo, w = offs[n], widths[n]
                for c in range(4):
                    nc.vector.tensor_scalar(
                        out=x1cf[:, c, 8 + lo:8 + lo + w],
                        in0=bufC[:, c * SPU + 8 + lo:c * SPU + 8 + lo + w],
                        scalar1=float(2.0 ** KX), scalar2=None, op0=MUL)

            with scope("hw0l0"):
                highway_layer(bufA, bufB, 0, dr=True)
            with scope("hw0l1"):
                highway_layer(bufB, bufC, 1, do_cmask=True)

            # ---- conv1 (+res) ----
            def conv1_range(n0, n1):
              for n in range(n0, n1):
                lo, w = offs[n], widths[n]
                for m in range(4):
                    ps = pp.tile([128, CW], F32, tag="ps", name="ps")
                    i = 0
                    for k in range(3):
                        for qp in range(2):
                            j = ((m * 3 + k) * 2 + qp) * 2
                            nc.tensor.matmul(
                                out=ps[:, 0:w], lhsT=t_wc1[:, j:j + 2, :],
                                rhs=x1cf[:, 2 * qp:2 * qp + 2, 7 + lo + k:7 + lo + k + w],
                                start=(i == 0), stop=(i == 5),
                                perf_mode=mybir.MatmulPerfMode.DoubleRow)
                            i += 1
                    r_t = tp.tile([128, CW], BF16, tag="h", name="r_t", bufs=4)
                    nc.scalar.activation(out=r_t[:, 0:w], in_=ps[:, 0:w], func=RELU,
                                         bias=t_bc1[:, m:m + 1],
                                         scale=float(2.0 ** -(KX + kw)))
                    xs = bufC[:, m * SPU + 8 + lo:m * SPU + 8 + lo + w]
                    nc.vector.tensor_tensor(
                        out=bufA[:, m * SPU + 8 + lo:m * SPU + 8 + lo + w],
                        in0=r_t[:, 0:w], in1=xs, op=ADD)

            # Multi-pass pipeline: pool work of each pass drains into the
            # next pass's conv1 PE window (DVE/GPSIMD are idle there). Each
            # hw1l1 pass stops one chunk short of the pass boundary: the next
            # conv1 pass still needs x1c chunk b-1's last col (hw1l1 would
            # overwrite it with x2).
            PB = [0, max(1, NCH - 6), max(2, NCH - 4), max(3, NCH - 2), NCH]
            NP = len(PB) - 1
            deferred = []
            for i in range(NP):
                b0, b1 = PB[i], PB[i + 1]
                with scope(f"conv1_{i}"):
                    conv1_range(b0, b1)
                    for pn in deferred:
                        pool_chunk(bufC, bufA, pn)
                    deferred = []
                with scope(f"hw1l0_{i}"):
                    highway_layer(bufA, bufB, 2, n0=b0, n1=b1)
                h0 = b0 - 1 if i > 0 else 0
                h1 = b1 - 1 if i < NP - 1 else NCH
                with scope(f"hw1l1_{i}"):
                    highway_layer(bufB, bufC, 3, do_pool=True, msel=bufA,
                                  n0=h0, n1=h1, pool_sink=deferred)
            with scope("pool_tail"):
                for pn in deferred:
                    pool_chunk(bufC, bufA, pn, eng_add=nc.gpsimd)

            # ---- projection over all stream cols ----
            ctx = scope("proj"); ctx.__enter__()
            for n in range(NCH):
                lo, w = offs[n], widths[n]
                hi = lo + w
                for m in range(4):
                    ps = pp.tile([128, CW], F32, tag="ps", name="ps")
                    for q in range(4):
                        nc.tensor.matmul(
                            out=ps[:, 0:w], lhsT=t_wpr[:, q * WED + m * 128:q * WED + (m + 1) * 128],
                            rhs=bufA[:, q * SPU + 8 + lo:q * SPU + 8 + hi],
                            start=(q == 0), stop=(q == 3))
                    o_t = tp.tile([128, CW], BF16, tag="o", name="o_t", bufs=3)
                    if n == NCH - 1 and m % 2 == 1:
                        # last chunk: split evacs across ACT and (idle) DVE to
                        # shorten the post-matmul tail
                        nc.vector.tensor_scalar(out=o_t[:, 0:w], in0=ps[:, 0:w],
                                                scalar1=t_bpr[:, m:m + 1],
                                                scalar2=None, op0=ADD)
                    else:
                        nc.scalar.activation(out=o_t[:, 0:w], in_=ps[:, 0:w], func=IDEN,
                                             bias=t_bpr[:, m:m + 1], scale=1.0)
                    dq = nc.sync if (n * 4 + m) % 2 == 0 else nc.scalar
                    dq.dma_start(out=out[m * 128:(m + 1) * 128, lo:hi], in_=o_t[:, 0:w])
            ctx.__exit__(None, None, None)

    nc.compile()
    return nc


def _prep_inputs(inputs):
    """Host-side: pack + shard + convert to the kernel's DRAM layouts."""
    byte_tokens = np.asarray(inputs["byte_tokens"], np.int64)
    bpe_mask = np.asarray(inputs["bpe_mask"], bool)
    pool_lengths = np.asarray(inputs["pool_lengths"], np.int64)
    tok_emb = np.asarray(inputs["tok_emb"], np.float32)

    cores, widths, S, (pl, cum, starts, src) = _plan_packing(pool_lengths)

    def bf(x):
        return np.ascontiguousarray(np.asarray(x, np.float32).astype(_BF16_NP))

    conv0_W = np.asarray(inputs["conv0_W"], np.float32)   # [3,128,512]
    conv1_W = np.asarray(inputs["conv1_W"], np.float32)   # [3,512,512]
    hw0_W = np.asarray(inputs["hw0_W"], np.float32)       # [2,1024,512]
    hw1_W = np.asarray(inputs["hw1_W"], np.float32)
    proj_W = np.asarray(inputs["proj_W"], np.float32)     # [512,512]

    w_c0 = bf(conv0_W.transpose(1, 0, 2).reshape(128, 3 * WED))
    kw = int(np.floor(np.log2(128.0 / max(np.abs(conv1_W).max(), 1e-30))))
    w_c1 = np.empty((128, 48, 128), np.float32)
    for m in range(4):
        for k in range(3):
            for q in range(4):
                j = ((m * 3 + k) * 2 + (q // 2)) * 2 + (q % 2)
                w_c1[:, j, :] = conv1_W[k, q * 128:(q + 1) * 128, m * 128:(m + 1) * 128]
    w_c1 = np.ascontiguousarray((w_c1 * 2.0 ** kw).astype(_F8_NP))
    whw = np.empty((128, 16, 1024), np.float32)
    for bl, (blk, lay) in enumerate(((hw0_W, 0), (hw0_W, 1), (hw1_W, 0), (hw1_W, 1))):
        wt = blk[lay].T  # [512, 1024]
        for q in range(4):
            whw[:, bl * 4 + q, :] = wt[q * 128:(q + 1) * 128]
    w_hw = bf(whw.reshape(128, 16 * 1024))
    kwh0 = int(np.floor(np.log2(128.0 / max(np.abs(hw0_W[0]).max(), 1e-30))))
    w_hw0 = np.empty((128, 32, 128), np.float32)
    for m in range(8):
        for q in range(4):
            w_hw0[:, m * 4 + q, :] = hw0_W[0, m * 128:(m + 1) * 128, q * 128:(q + 1) * 128].T
    w_hw0 = np.ascontiguousarray((w_hw0 * 2.0 ** kwh0).astype(_F8_NP))
    w_pr = bf(proj_W.T.reshape(4, 128, WED).transpose(1, 0, 2).reshape(128, 4 * WED))

    def colchunks(b):  # [512] -> [128, 4]
        return np.ascontiguousarray(np.asarray(b, np.float32).reshape(4, 128).T)

    b_c0 = colchunks(inputs["conv0_b"])
    b_c1 = colchunks(inputs["conv1_b"])
    bhw = np.empty((128, 4, 8), np.float32)
    for bl, (blk, lay) in enumerate((("hw0_b", 0), ("hw0_b", 1), ("hw1_b", 0), ("hw1_b", 1))):
        b = np.asarray(inputs[blk], np.float32)[lay]      # [1024]
        bhw[:, bl, 0:4] = b[:512].reshape(4, 128).T
        bhw[:, bl, 4:8] = b[512:1024].reshape(4, 128).T
    b_hw = np.ascontiguousarray(bhw.reshape(128, 32))
    b_pr = colchunks(inputs["proj_b"])

    emb_lhs = np.zeros((128, 3 * 128), np.float32)
    emb_lhs[:, 0:128] = tok_emb[0:128]
    emb_lhs[:, 128:256] = tok_emb[128:256]
    emb_lhs[0:8, 256:384] = tok_emb[256:264]
    emb_lhs = bf(emb_lhs)
    emb_row4 = bf(tok_emb[BPE_MASK_IDX:BPE_MASK_IDX + 1, :])  # [1, 128]
    iota_c = np.empty((128, 3), np.float32)
    p = np.arange(128)
    iota_c[:, 0] = p
    iota_c[:, 1] = 128 + p
    iota_c[:, 2] = _enc_ids(256 + p)   # only partitions 0..7 used

    shared = dict(emb_lhs=emb_lhs, emb_row4=emb_row4, iota_c=iota_c,
                  w_c0=w_c0, w_c1=w_c1, w_hw=w_hw, w_hw0=w_hw0, w_pr=w_pr,
                  b_c0=b_c0, b_c1=b_c1, b_hw=b_hw, b_pr=b_pr)

    in_maps = []
    meta = []
    for core in range(N_CORES):
        segs, _wr = cores[core]
        tok = np.zeros(S, np.float32)
        bpe = np.zeros(S, np.float32)
        a1 = np.full(S, NEG_BIG, np.float32)
        a2 = np.full(S, NEG_BIG, np.float32)
        cmk = np.zeros(S, np.float32)
        wrows, wcols = [], []
        pos = 0
        for (b, lw0, lw1, t0, t1) in segs:
            if t0 > 0:
                hl = min(2, t0)
                tok[pos:pos + hl] = _enc_ids(byte_tokens[b, t0 - hl:t0])
                bpe[pos:pos + hl] = bpe_mask[b, t0 - hl:t0]
                cmk[pos:pos + hl] = 1.0
                pos += hl
            body = pos
            nb = t1 - t0
            tok[pos:pos + nb] = _enc_ids(byte_tokens[b, t0:t1])
            bpe[pos:pos + nb] = bpe_mask[b, t0:t1]
            cmk[pos:pos + nb] = 1.0
            lw = np.arange(lw0, lw1)
            ln = pl[b, lw0:lw1]
            wst = starts[b, lw0:lw1] - t0 + body
            nz = ln > 0
            a1[wst[nz]] = np.where(ln[nz] > 1, 0.0, NEG_BIG)
            a2[wst[nz]] = np.where(ln[nz] > 2, 0.0, NEG_BIG)
            wrows.append(b * NW + lw[nz])
            wcols.append(wst[nz])
            pos += nb
            if t1 == int(src[b]):
                cmk[pos] = 1.0      # gap1: reference position src_len
                pos += 2
            else:
                hr = min(2, int(src[b]) - t1)
                tok[pos:pos + hr] = _enc_ids(byte_tokens[b, t1:t1 + hr])
                bpe[pos:pos + hr] = bpe_mask[b, t1:t1 + hr]
                cmk[pos] = 1.0
                pos += hr
        assert pos <= S, (pos, S)

        m = dict(shared)
        m["tok_bc"] = np.ascontiguousarray(
            np.broadcast_to(tok.astype(_BF16_NP), (128, S)))
        m["bpe_row"] = bpe.astype(_BF16_NP).reshape(1, S)
        am = np.concatenate([a1, a2])
        am = np.where(am < 0, NEG_F8, 0.0).astype(_F8_NP)
        m["a_msk"] = np.ascontiguousarray(np.broadcast_to(am, (128, 2 * S)))
        m["c_msk"] = np.ascontiguousarray(
            np.broadcast_to((cmk * 2.0 ** KX).astype(_BF16_NP), (128, S)))
        in_maps.append(m)
        meta.append((np.concatenate(wrows) if wrows else np.empty(0, np.int64),
                     np.concatenate(wcols) if wcols else np.empty(0, np.int64)))
    return in_maps, (meta, widths, kw, kwh0)


def kernel(**inputs) -> np.ndarray:
    from concourse.bass_utils import run_bass_kernel_spmd

    in_maps, (meta, widths, kw, kwh0) = _prep_inputs(inputs)
    key = (widths, kw, kwh0)
    if _CACHE.get("key") != key:
        _CACHE["nc"] = _build_program(widths, kw, kwh0)
        _CACHE["key"] = key
    nc = _CACHE["nc"]

    res = run_bass_kernel_spmd(nc, in_maps, list(range(N_CORES)))

    proj_b = np.asarray(inputs["proj_b"], np.float32)
    full = np.empty((BSZ * NW, WED), np.float32)
    full[:] = proj_b
    for core in range(N_CORES):
        o = np.asarray(res.results[core]["out"], np.float32)  # [512, S]
        rows, cols = meta[core]
        if len(rows):
            full[rows] = o[:, cols].T
    return full.reshape(BSZ, NW, WED)
